# revision 33
# baseline (speedup 1.0000x reference)
"""CapsuleLinear (dynamic routing) Trainium2 kernel, v3.

Reference computes priors = einsum('oli,bni->bonl', W, x) (302MB) then runs 3
routing iterations. We never materialize priors. Key algebraic identity: the
logits update needs wv = W^T(squash(W s)) = f * (W^T W) s = f * G s with
f = ||Ws||/(1+||Ws||^2) and ns = ||Ws||^2 = <s, G s>.  G[o] = W[o]^T W[o] is a
static per-capsule 32x32 matrix, precomputed on the host.  So iterations 0-1
never form v at all; only the final iteration applies W itself:

  per routing iteration r:
    probs[n,o]   = softmax_o(logits[n,o])          (exp on ACT, Z on DVE/GPS)
    s[o,i]       = sum_n probs[n,o] x[n,i]         (PE matmul, contract n)
    q[o,i]       = sum_i' G[o,i,i'] s[o,i']        (DVE/GPS bf16 mul+reduce)
    ns[o]        = sum_i s[o,i] q[o,i]             (tiny)
    wv[o,i]      = f(ns) * q[o,i]                  (tiny scale)
    logits[n,o] += sum_i x[n,i] wv[o,i]            (PE matmul, contract i)
  final r: v = squash(W s) via wli, DMA out.

Iteration 0 has uniform probs -> s0 = colsum(x)/64, so wv0 depends only on
colmean(x) and W: computed on the host (~0.5% of FLOPs) and shipped as a 16KB
wvT tile.  Device iteration 0 is just the 36 delta matmuls.

Sharding: data-parallel over batch N=32 -> 4 batches per core on 8 cores.
Weight (64,32,32) replicated. No collectives.

sqrt(ns) is computed as exp(0.5*ln(ns)) so the whole kernel uses one ACT
table set (natural_log_exp_and_others) - no 1.3us table switches.

Per-core layouts:
  x_sb  [128(p), 4(b), 9(c), 32(i)]   x[b, c*128+p, i]          bf16
  xt_sb [32(i), 4(b), 9(c), 128(p)]   host-transposed x         bf16
  gw    [128(b2*64+o), 2, 32, 32]     [G | wli] pair-replicated bf16
  logits PSUM [128(p), 2(b2), 9(c), 64(o)] x2 halves, fp32
  pair tiles [128(b2*64+o), 2(pair), ...] 2 batches stacked on partitions
Engine split: pair-0 reduces + Z(b0,b1) + xr(b2,b3) on DVE; pair-1 reduces +
Z(b2,b3) + xr(b0,b1) on GPSIMD; exp/copies/ln/exp on ACT.
"""

import os
import sys

for _p in ("/opt/trn_rl_repo",):
    if _p not in sys.path and os.path.isdir(_p):
        sys.path.insert(0, _p)

import numpy as np

import concourse.bacc as bacc
import concourse.bass as bass
import concourse.tile as tile
from concourse import mybir
from concourse.bass_utils import run_bass_kernel_spmd

N_TOT, N_CAPS, I_LEN = 32, 1152, 32
O_CAPS, L_LEN = 64, 32
NCORES = 8
B = N_TOT // NCORES  # 4 batches per core
C = N_CAPS // 128    # 9 chunks of 128 input capsules
PAIRS = B // 2
FP = mybir.dt.float32
BF = mybir.dt.bfloat16
Exp = mybir.ActivationFunctionType.Exp
Ln = mybir.ActivationFunctionType.Ln
X = mybir.AxisListType.X
MUL = mybir.AluOpType.mult


def build_nc():
    nc = bacc.Bacc("TRN2", target_bir_lowering=False, debug=True)
    x_nat_d = nc.dram_tensor("x_nat", [128, B, I_LEN, C + 1], BF, kind="ExternalInput")
    xt_d = nc.dram_tensor("xt", [I_LEN, B, C, 128], BF, kind="ExternalInput")
    # [G | wli], both pair-replicated to 128 rows on the host (no on-chip
    # replication chain on the critical path)
    gw_d = nc.dram_tensor("gw", [128, 2, L_LEN, I_LEN], BF, kind="ExternalInput")
    ident_d = nc.dram_tensor("ident", [128, 128], FP, kind="ExternalInput")
    wvt0_d = nc.dram_tensor("wvt0", [I_LEN, PAIRS, 128], BF, kind="ExternalInput")
    out_d = nc.dram_tensor("out", [128, PAIRS, L_LEN], FP, kind="ExternalOutput")

    with tile.TileContext(nc) as tc:
        with (
            tc.tile_pool(name="main", bufs=1) as pool,
            tc.tile_pool(name="psum", bufs=1, space="PSUM") as psum,
        ):
            x_sb = pool.tile([128, B, I_LEN, C + 1], BF)
            xt_sb = pool.tile([I_LEN, B, C, 128], BF)
            ph = pool.tile([128, B, C, O_CAPS // 2], BF)
            gw_sb = pool.tile([128, 2, L_LEN, I_LEN], BF)
            g_sb = gw_sb[:, 0]            # [128, 32(i), 32(i')]
            wli_sb = gw_sb[:, 1]          # [128, 32(l), 32(i)]
            ident = pool.tile([128, 128], FP)
            shift = pool.tile([128, 1], FP)
            s_sb = pool.tile([128, PAIRS, I_LEN], BF)
            pexp = pool.tile([128, B, C, O_CAPS], BF)
            zsum = pool.tile([128, B, C + 1], BF)
            rinv = pool.tile([128, B, C + 1], BF)
            xr = pool.tile([128, B, I_LEN, C + 1], BF)
            qph = pool.tile([128, PAIRS, L_LEN, I_LEN // 2], BF)
            qprod = pool.tile([128, PAIRS, L_LEN, I_LEN], BF)
            q_raw = pool.tile([128, PAIRS, I_LEN], FP)
            nsprod = pool.tile([128, PAIRS, I_LEN], BF)
            ns = pool.tile([128, PAIRS], FP)
            lnns = pool.tile([128, PAIRS], FP)
            vnorm = pool.tile([128, PAIRS], FP)
            denom = pool.tile([128, PAIRS], FP)
            rden = pool.tile([128, PAIRS], FP)
            fsc = pool.tile([128, PAIRS], FP)
            wv_f = pool.tile([128, PAIRS, I_LEN], FP)
            wvt_sb = pool.tile([I_LEN, PAIRS, 128], BF)
            v_raw = pool.tile([128, PAIRS, L_LEN], BF)
            v_out = pool.tile([128, PAIRS, L_LEN], FP)

            logits_ps = [
                psum.tile([128, 2, C, O_CAPS], FP, name=f"logits_ps{h}", tag=f"lg{h}")
                for h in range(2)
            ]
            u_ps = [
                psum.tile([128, 512], FP, name=f"u_ps{t}", tag=f"u_ps{t}")
                for t in range(PAIRS)
            ]
            s_ps = [u_ps[t][:, 0:I_LEN] for t in range(PAIRS)]
            wvt_ps = [u_ps[t][0:I_LEN, 128:256] for t in range(PAIRS)]

            # --- input DMAs across 3 queues. xt gates iter-0's delta:
            # split in thirds. x feeds iter-1's xr.
            nc.gpsimd.dma_start(out=wvt_sb[:], in_=wvt0_d[:])
            nc.sync.dma_start(out=xt_sb[:, 0:2], in_=xt_d[:, 0:2])
            nc.gpsimd.dma_start(out=xt_sb[:, 2:3], in_=xt_d[:, 2:3])
            nc.scalar.dma_start(out=xt_sb[:, 3:4], in_=xt_d[:, 3:4])
            nc.sync.dma_start(out=x_sb[:, 0:2], in_=x_nat_d[:, 0:2])
            nc.gpsimd.dma_start(out=x_sb[:, 2:4], in_=x_nat_d[:, 2:4])
            nc.scalar.dma_start(out=gw_sb[:], in_=gw_d[:])
            nc.scalar.dma_start(out=ident[:], in_=ident_d[:])
            nc.vector.memset(shift[:], -40.0)
            nc.vector.memset(zsum[:], 1.0)

            # iter-0 delta straight from the host-provided wvt0, pair 1
            # first (matches the pair order of the iteration loop below).
            # One start/stop per 2KB psum bank (8 chunks per bank).
            for t in (1, 0):
                for b2 in range(2):
                    b = 2 * t + b2
                    for c in range(C):
                        k = b2 * C + c
                        nc.tensor.matmul(
                            out=logits_ps[t][:, b2, c, :],
                            lhsT=xt_sb[:, b, c, :],
                            rhs=wvt_sb[:, t, b2 * 64 : (b2 + 1) * 64],
                            start=(k % 8 == 0),
                            stop=(k % 8 == 7 or k == 2 * C - 1),
                        )

            # Iterations 1..2. Emission follows execution time-order to
            # avoid FIFO head-of-line blocking: both pairs' softmax fronts
            # first (pair 1, then pair 0), then both capsule chains. Pair
            # 0's front only waits on its own previous-iteration delta.
            def front(r, t):
                ba, bb = 2 * t, 2 * t + 1
                nc.scalar.activation(
                    out=pexp[:, ba], in_=logits_ps[t][:, 0], func=Exp, bias=shift[:]
                )
                nc.scalar.activation(
                    out=pexp[:, bb], in_=logits_ps[t][:, 1], func=Exp, bias=shift[:]
                )
                with nc.allow_low_precision(reason="bf16 softmax"):
                    nc.vector.reduce_sum(out=zsum[:, ba, 0:C], in_=pexp[:, ba], axis=X)
                    nc.vector.reciprocal(out=rinv[:, ba], in_=zsum[:, ba])  # incl pad lane
                nc.vector.tensor_mul(
                    out=xr[:, ba],
                    in0=x_sb[:, ba],
                    in1=rinv[:, ba].unsqueeze(1).broadcast_to((128, I_LEN, C + 1)),
                )
                nc.gpsimd.tensor_add(
                    out=ph[:, bb],
                    in0=pexp[:, bb, :, 0:32],
                    in1=pexp[:, bb, :, 32:64],
                )
                for c in range(C):
                    nc.tensor.matmul(
                        out=s_ps[t][0:64, :],
                        lhsT=pexp[:, ba, c, :],
                        rhs=xr[:, ba, :, c],
                        start=(c == 0),
                        stop=(c == C - 1),
                        tile_position=(0, 0),
                    )
                with nc.allow_low_precision(reason="bf16 softmax"):
                    nc.vector.reduce_sum(out=zsum[:, bb, 0:C], in_=ph[:, bb], axis=X)
                    nc.vector.reciprocal(out=rinv[:, bb], in_=zsum[:, bb])
                nc.vector.tensor_mul(
                    out=xr[:, bb],
                    in0=x_sb[:, bb],
                    in1=rinv[:, bb].unsqueeze(1).broadcast_to((128, I_LEN, C + 1)),
                )
                for c in range(C):
                    nc.tensor.matmul(
                        out=s_ps[t][64:128, :],
                        lhsT=pexp[:, bb, c, :],
                        rhs=xr[:, bb, :, c],
                        start=(c == 0),
                        stop=(c == C - 1),
                        tile_position=(0, 64),
                    )

            def capsule(r, t):
                ts = slice(t, t + 1)
                nc.scalar.copy(out=s_sb[:, t, :], in_=s_ps[t][:])
                # q = G s (iter 1) or v_raw = W s (final); q_raw fp32.
                nc.vector.tensor_mul(
                    out=qprod[:, t],
                    in0=(wli_sb if r == 2 else g_sb)[:],
                    in1=s_sb[:, t, :].unsqueeze(1).broadcast_to((128, L_LEN, I_LEN)),
                )
                nc.gpsimd.tensor_add(
                    out=qph[:, t],
                    in0=qprod[:, t, :, 0 : I_LEN // 2],
                    in1=qprod[:, t, :, I_LEN // 2 : I_LEN],
                )
                nc.vector.reduce_sum(out=q_raw[:, t, :], in_=qph[:, t], axis=X)
                if r == 1:
                    nc.vector.tensor_mul(
                        out=nsprod[:, t], in0=s_sb[:, t], in1=q_raw[:, t]
                    )
                    nc.vector.reduce_sum(
                        out=ns[:, ts], in_=nsprod[:, t].unsqueeze(1), axis=X
                    )
                else:
                    # r2: ns = ||v||^2 via ACT square+accumulate
                    nc.scalar.activation(
                        out=nsprod[:, t], in_=q_raw[:, t],
                        func=mybir.ActivationFunctionType.Square,
                        accum_out=ns[:, ts],
                    )
                nc.scalar.activation(out=lnns[:, ts], in_=ns[:, ts], func=Ln)
                nc.scalar.activation(
                    out=vnorm[:, ts], in_=lnns[:, ts], func=Exp, scale=0.5
                )
                nc.vector.tensor_scalar_add(out=denom[:, ts], in0=ns[:, ts], scalar1=1.0)
                nc.vector.reciprocal(out=rden[:, ts], in_=denom[:, ts])
                if r == 2:
                    nc.vector.scalar_tensor_tensor(
                        out=v_out[:, t],
                        in0=q_raw[:, t],
                        scalar=vnorm[:, ts],
                        in1=rden[:, ts].broadcast_to((128, L_LEN)),
                        op0=MUL,
                        op1=MUL,
                    )
                    nc.sync.dma_start(out=out_d[:, t], in_=v_out[:, t])
                    return
                nc.vector.scalar_tensor_tensor(
                    out=wv_f[:, t],
                    in0=q_raw[:, t],
                    scalar=vnorm[:, ts],
                    in1=rden[:, ts].broadcast_to((128, I_LEN)),
                    op0=MUL,
                    op1=MUL,
                )
                nc.tensor.transpose(
                    out=wvt_ps[t][:], in_=wv_f[:, t, :], identity=ident[:]
                )
                nc.scalar.copy(out=wvt_sb[:, t, :], in_=wvt_ps[t][:])
                # delta; accumulate onto surviving has_written bits from
                # iter 0 (start/stop must be explicit False: None = auto).
                for b2 in range(2):
                    b = 2 * t + b2
                    for c in range(C):
                        nc.tensor.matmul(
                            out=logits_ps[t][:, b2, c, :],
                            lhsT=xt_sb[:, b, c, :],
                            rhs=wvt_sb[:, t, b2 * 64 : (b2 + 1) * 64],
                            start=False,
                            stop=False,
                            skip_group_check=True,
                        )

            for r in (1, 2):
                front(r, 1)
                front(r, 0)
                capsule(r, 1)
                capsule(r, 0)
    return nc


_NC = None


def get_nc():
    global _NC
    if _NC is None:
        _NC = build_nc()
    return _NC


def to_bf16(a):
    import ml_dtypes

    return a.astype(ml_dtypes.bfloat16)


def make_in_maps(x, weight):
    x = np.ascontiguousarray(x, dtype=np.float32)
    w = np.ascontiguousarray(weight, dtype=np.float32)  # [64, 32(l), 32(i)]
    G = np.einsum("oli,olj->oij", w, w)  # [64, 32(i), 32(i')]
    gw = np.stack([G, w], axis=1)  # [64, 2, 32, 32]
    gw = np.tile(gw, (2, 1, 1, 1))  # pair-replicated [128, 2, 32, 32]
    ident = np.eye(128, dtype=np.float32)
    in_maps = []
    for core in range(NCORES):
        xs = x[core * B : (core + 1) * B]  # [B, 1152, 32]
        xc = xs.reshape(B, C, 128, I_LEN)
        x_nat = xc.transpose(2, 0, 3, 1)  # [128, B, 32(i), C]
        x_nat = np.concatenate([x_nat, np.zeros_like(x_nat[..., :1])], axis=-1)
        xt = np.ascontiguousarray(xc.transpose(3, 0, 1, 2))  # [32, B, C, 128]
        # iteration-0: probs uniform -> s0 = colsum(x)/64; wv0 = f0 * G s0
        s0 = xs.sum(axis=1) / O_CAPS  # [B, 32]
        q0 = np.einsum("oij,bj->boi", G, s0)  # [B, 64, 32]
        ns0 = np.einsum("boi,bi->bo", q0, s0)  # [B, 64]
        f0 = np.sqrt(ns0) / (1.0 + ns0)
        wv0 = f0[:, :, None] * q0  # [B, 64, 32]
        # wvt0[i, t, b2*64+o] = wv0[2t+b2, o, i]
        wvt0 = np.empty((I_LEN, PAIRS, 2, O_CAPS), dtype=np.float32)
        for t in range(PAIRS):
            for b2 in range(2):
                wvt0[:, t, b2, :] = wv0[2 * t + b2].T
        wvt0 = wvt0.reshape(I_LEN, PAIRS, 128)
        in_maps.append(
            {
                "x_nat": to_bf16(x_nat),
                "xt": to_bf16(xt),
                "gw": to_bf16(gw),
                "ident": ident,
                "wvt0": to_bf16(wvt0),
            }
        )
    return in_maps


def assemble(results):
    outs = []
    for core in range(len(results)):
        o = np.asarray(results[core]["out"], dtype=np.float32)  # [128, PAIRS, 32]
        # v[b2*64+o, t, l] -> [b=2t+b2, o, l]
        o = o.reshape(2, O_CAPS, PAIRS, L_LEN).transpose(2, 0, 1, 3)  # [t, b2, o, l]
        outs.append(o.reshape(B, O_CAPS, L_LEN))
    return np.concatenate(outs, axis=0)


def _pin_act_table_set(nc):
    """Make Exp and Ln resolve to the one table set containing both
    (natural_log_exp_and_others), so the whole kernel runs on a single
    ACT table load."""
    from concourse.hw_specs import get_activation_tables

    tabs = get_activation_tables(nc.m.arch)
    for name, funcs in tabs.items():
        if name != "natural_log_exp_and_others":
            funcs.discard(Exp)
            funcs.discard(Ln)
            funcs.discard(mybir.ActivationFunctionType.Square)
            funcs.discard(mybir.ActivationFunctionType.Copy)
            funcs.discard(mybir.ActivationFunctionType.Identity)


def run(x, weight, trace=False):
    nc = get_nc()
    if not nc.is_finalized():
        _pin_act_table_set(nc)
        nc.finalize()
    res = run_bass_kernel_spmd(nc, make_in_maps(x, weight), list(range(NCORES)), trace=trace)
    return assemble(res.results), res


def kernel(x, weight):
    out, _ = run(x, weight)
    return out


# revision 34
# speedup vs baseline: 1.0248x; 1.0248x over previous
"""CapsuleLinear (dynamic routing) Trainium2 kernel, v3.

Reference computes priors = einsum('oli,bni->bonl', W, x) (302MB) then runs 3
routing iterations. We never materialize priors. Key algebraic identity: the
logits update needs wv = W^T(squash(W s)) = f * (W^T W) s = f * G s with
f = ||Ws||/(1+||Ws||^2) and ns = ||Ws||^2 = <s, G s>.  G[o] = W[o]^T W[o] is a
static per-capsule 32x32 matrix, precomputed on the host.  So iterations 0-1
never form v at all; only the final iteration applies W itself:

  per routing iteration r:
    probs[n,o]   = softmax_o(logits[n,o])          (exp on ACT, Z on DVE/GPS)
    s[o,i]       = sum_n probs[n,o] x[n,i]         (PE matmul, contract n)
    q[o,i]       = sum_i' G[o,i,i'] s[o,i']        (DVE/GPS bf16 mul+reduce)
    ns[o]        = sum_i s[o,i] q[o,i]             (tiny)
    wv[o,i]      = f(ns) * q[o,i]                  (tiny scale)
    logits[n,o] += sum_i x[n,i] wv[o,i]            (PE matmul, contract i)
  final r: v = squash(W s) via wli, DMA out.

Iteration 0 has uniform probs -> s0 = colsum(x)/64, so wv0 depends only on
colmean(x) and W: computed on the host (~0.5% of FLOPs) and shipped as a 16KB
wvT tile.  Device iteration 0 is just the 36 delta matmuls.

Sharding: data-parallel over batch N=32 -> 4 batches per core on 8 cores.
Weight (64,32,32) replicated. No collectives.

sqrt(ns) is computed as exp(0.5*ln(ns)) so the whole kernel uses one ACT
table set (natural_log_exp_and_others) - no 1.3us table switches.

Per-core layouts:
  x_sb  [128(p), 4(b), 9(c), 32(i)]   x[b, c*128+p, i]          bf16
  xt_sb [32(i), 4(b), 9(c), 128(p)]   host-transposed x         bf16
  gw    [128(b2*64+o), 2, 32, 32]     [G | wli] pair-replicated bf16
  logits PSUM [128(p), 2(b2), 9(c), 64(o)] x2 halves, fp32
  pair tiles [128(b2*64+o), 2(pair), ...] 2 batches stacked on partitions
Engine split: pair-0 reduces + Z(b0,b1) + xr(b2,b3) on DVE; pair-1 reduces +
Z(b2,b3) + xr(b0,b1) on GPSIMD; exp/copies/ln/exp on ACT.
"""

import os
import sys

for _p in ("/opt/trn_rl_repo",):
    if _p not in sys.path and os.path.isdir(_p):
        sys.path.insert(0, _p)

import numpy as np

import concourse.bacc as bacc
import concourse.bass as bass
import concourse.tile as tile
from concourse import mybir
from concourse.bass_utils import run_bass_kernel_spmd

N_TOT, N_CAPS, I_LEN = 32, 1152, 32
O_CAPS, L_LEN = 64, 32
NCORES = 8
B = N_TOT // NCORES  # 4 batches per core
C = N_CAPS // 128    # 9 chunks of 128 input capsules
PAIRS = B // 2
FP = mybir.dt.float32
BF = mybir.dt.bfloat16
Exp = mybir.ActivationFunctionType.Exp
Ln = mybir.ActivationFunctionType.Ln
X = mybir.AxisListType.X
MUL = mybir.AluOpType.mult


def build_nc():
    nc = bacc.Bacc("TRN2", target_bir_lowering=False, debug=True)
    x_nat_d = nc.dram_tensor("x_nat", [128, B, I_LEN, C + 1], BF, kind="ExternalInput")
    xt_d = nc.dram_tensor("xt", [I_LEN, B, C, 128], BF, kind="ExternalInput")
    # [G | wli], both pair-replicated to 128 rows on the host (no on-chip
    # replication chain on the critical path)
    gw_d = nc.dram_tensor("gw", [128, 2, L_LEN, I_LEN], BF, kind="ExternalInput")
    ident_d = nc.dram_tensor("ident", [128, 128], FP, kind="ExternalInput")
    wvt0_d = nc.dram_tensor("wvt0", [I_LEN, PAIRS, 128], BF, kind="ExternalInput")
    out_d = nc.dram_tensor("out", [128, PAIRS, L_LEN], FP, kind="ExternalOutput")

    with tile.TileContext(nc) as tc:
        with (
            tc.tile_pool(name="main", bufs=1) as pool,
            tc.tile_pool(name="psum", bufs=1, space="PSUM") as psum,
        ):
            x_sb = pool.tile([128, B, I_LEN, C + 1], BF)
            xt_sb = pool.tile([I_LEN, B, C, 128], BF)
            ph = pool.tile([128, B, C, O_CAPS // 2], BF)
            gw_sb = pool.tile([128, 2, L_LEN, I_LEN], BF)
            g_sb = gw_sb[:, 0]            # [128, 32(i), 32(i')]
            wli_sb = gw_sb[:, 1]          # [128, 32(l), 32(i)]
            ident = pool.tile([128, 128], FP)
            shift = pool.tile([128, 1], FP)
            s_sb = pool.tile([128, PAIRS, I_LEN], BF)
            pexp = pool.tile([128, B, C, O_CAPS], BF)
            zsum = pool.tile([128, B, C + 1], BF)
            rinv = pool.tile([128, B, C + 1], BF)
            xr = pool.tile([128, B, I_LEN, C + 1], BF)
            qph = pool.tile([128, PAIRS, L_LEN, I_LEN // 2], BF)
            qprod = pool.tile([128, PAIRS, L_LEN, I_LEN], BF)
            q_raw = pool.tile([128, PAIRS, I_LEN], FP)
            nsprod = pool.tile([128, PAIRS, I_LEN], BF)
            ns = pool.tile([128, PAIRS], FP)
            lnns = pool.tile([128, PAIRS], FP)
            vnorm = pool.tile([128, PAIRS], FP)
            denom = pool.tile([128, PAIRS], FP)
            rden = pool.tile([128, PAIRS], FP)
            fsc = pool.tile([128, PAIRS], FP)
            wv_f = pool.tile([128, PAIRS, I_LEN], FP)
            wvt_sb = pool.tile([I_LEN, PAIRS, 128], BF)
            v_raw = pool.tile([128, PAIRS, L_LEN], BF)
            v_out = pool.tile([128, PAIRS, L_LEN], FP)

            logits_ps = [
                psum.tile([128, 2, C, O_CAPS], FP, name=f"logits_ps{h}", tag=f"lg{h}")
                for h in range(2)
            ]
            u_ps = [
                psum.tile([128, 512], FP, name=f"u_ps{t}", tag=f"u_ps{t}")
                for t in range(PAIRS)
            ]
            s_ps = [u_ps[t][:, 0:I_LEN] for t in range(PAIRS)]
            wvt_ps = [u_ps[t][0:I_LEN, 128:256] for t in range(PAIRS)]

            # --- input DMAs across 3 queues. xt gates iter-0's delta:
            # split in thirds. x feeds iter-1's xr.
            nc.gpsimd.dma_start(out=wvt_sb[:], in_=wvt0_d[:])
            nc.sync.dma_start(out=xt_sb[:, 0:2], in_=xt_d[:, 0:2])
            nc.gpsimd.dma_start(out=xt_sb[:, 2:3], in_=xt_d[:, 2:3])
            nc.scalar.dma_start(out=xt_sb[:, 3:4], in_=xt_d[:, 3:4])
            nc.sync.dma_start(out=x_sb[:, 0:2], in_=x_nat_d[:, 0:2])
            nc.gpsimd.dma_start(out=x_sb[:, 2:4], in_=x_nat_d[:, 2:4])
            nc.scalar.dma_start(out=gw_sb[:], in_=gw_d[:])
            nc.scalar.dma_start(out=ident[:], in_=ident_d[:])
            nc.vector.memset(shift[:], -40.0)
            nc.vector.memset(zsum[:], 1.0)

            # iter-0 delta straight from the host-provided wvt0, pair 1
            # first (matches the pair order of the iteration loop below).
            # One start/stop per 2KB psum bank (8 chunks per bank).
            for t in (1, 0):
                for b2 in range(2):
                    b = 2 * t + b2
                    for c in range(C):
                        k = b2 * C + c
                        nc.tensor.matmul(
                            out=logits_ps[t][:, b2, c, :],
                            lhsT=xt_sb[:, b, c, :],
                            rhs=wvt_sb[:, t, b2 * 64 : (b2 + 1) * 64],
                            start=(k % 8 == 0),
                            stop=(k % 8 == 7 or k == 2 * C - 1),
                        )

            # Iterations 1..2. Emission follows execution time-order to
            # avoid FIFO head-of-line blocking: both pairs' softmax fronts
            # first (pair 1, then pair 0), then both capsule chains. Pair
            # 0's front only waits on its own previous-iteration delta.
            def front(r, t):
                ba, bb = 2 * t, 2 * t + 1
                nc.scalar.activation(
                    out=pexp[:, ba], in_=logits_ps[t][:, 0], func=Exp, bias=shift[:]
                )
                nc.scalar.activation(
                    out=pexp[:, bb], in_=logits_ps[t][:, 1], func=Exp, bias=shift[:]
                )
                with nc.allow_low_precision(reason="bf16 softmax"):
                    nc.vector.reduce_sum(out=zsum[:, ba, 0:C], in_=pexp[:, ba], axis=X)
                    nc.vector.reciprocal(out=rinv[:, ba], in_=zsum[:, ba])  # incl pad lane
                nc.vector.tensor_mul(
                    out=xr[:, ba],
                    in0=x_sb[:, ba],
                    in1=rinv[:, ba].unsqueeze(1).broadcast_to((128, I_LEN, C + 1)),
                )
                nc.gpsimd.tensor_add(
                    out=ph[:, bb],
                    in0=pexp[:, bb, :, 0:32],
                    in1=pexp[:, bb, :, 32:64],
                )
                for c in range(C):
                    nc.tensor.matmul(
                        out=s_ps[t][0:64, :],
                        lhsT=pexp[:, ba, c, :],
                        rhs=xr[:, ba, :, c],
                        start=(c == 0),
                        stop=(c == C - 1),
                        tile_position=(0, 0),
                    )
                with nc.allow_low_precision(reason="bf16 softmax"):
                    nc.vector.reduce_sum(out=zsum[:, bb, 0:C], in_=ph[:, bb], axis=X)
                    nc.vector.reciprocal(out=rinv[:, bb], in_=zsum[:, bb])
                nc.vector.tensor_mul(
                    out=xr[:, bb],
                    in0=x_sb[:, bb],
                    in1=rinv[:, bb].unsqueeze(1).broadcast_to((128, I_LEN, C + 1)),
                )
                for c in range(C):
                    nc.tensor.matmul(
                        out=s_ps[t][64:128, :],
                        lhsT=pexp[:, bb, c, :],
                        rhs=xr[:, bb, :, c],
                        start=(c == 0),
                        stop=(c == C - 1),
                        tile_position=(0, 64),
                    )

            def capsule(r, t):
                ts = slice(t, t + 1)
                nc.scalar.copy(out=s_sb[:, t, :], in_=s_ps[t][:])
                # q = G s (iter 1) or v_raw = W s (final); q_raw fp32.
                nc.vector.tensor_mul(
                    out=qprod[:, t],
                    in0=(wli_sb if r == 2 else g_sb)[:],
                    in1=s_sb[:, t, :].unsqueeze(1).broadcast_to((128, L_LEN, I_LEN)),
                )
                nc.vector.reduce_sum(out=q_raw[:, t, :], in_=qprod[:, t], axis=X)
                if r == 1:
                    nc.vector.tensor_mul(
                        out=nsprod[:, t], in0=s_sb[:, t], in1=q_raw[:, t]
                    )
                    nc.vector.reduce_sum(
                        out=ns[:, ts], in_=nsprod[:, t].unsqueeze(1), axis=X
                    )
                else:
                    # r2: ns = ||v||^2 via ACT square+accumulate
                    nc.scalar.activation(
                        out=nsprod[:, t], in_=q_raw[:, t],
                        func=mybir.ActivationFunctionType.Square,
                        accum_out=ns[:, ts],
                    )
                nc.scalar.activation(out=lnns[:, ts], in_=ns[:, ts], func=Ln)
                nc.scalar.activation(
                    out=vnorm[:, ts], in_=lnns[:, ts], func=Exp, scale=0.5
                )
                nc.vector.tensor_scalar_add(out=denom[:, ts], in0=ns[:, ts], scalar1=1.0)
                nc.vector.reciprocal(out=rden[:, ts], in_=denom[:, ts])
                if r == 2:
                    nc.vector.scalar_tensor_tensor(
                        out=v_out[:, t],
                        in0=q_raw[:, t],
                        scalar=vnorm[:, ts],
                        in1=rden[:, ts].broadcast_to((128, L_LEN)),
                        op0=MUL,
                        op1=MUL,
                    )
                    nc.sync.dma_start(out=out_d[:, t], in_=v_out[:, t])
                    return
                nc.vector.scalar_tensor_tensor(
                    out=wv_f[:, t],
                    in0=q_raw[:, t],
                    scalar=vnorm[:, ts],
                    in1=rden[:, ts].broadcast_to((128, I_LEN)),
                    op0=MUL,
                    op1=MUL,
                )
                nc.tensor.transpose(
                    out=wvt_ps[t][:], in_=wv_f[:, t, :], identity=ident[:]
                )
                nc.scalar.copy(out=wvt_sb[:, t, :], in_=wvt_ps[t][:])
                # delta; accumulate onto surviving has_written bits from
                # iter 0 (start/stop must be explicit False: None = auto).
                for b2 in range(2):
                    b = 2 * t + b2
                    for c in range(C):
                        nc.tensor.matmul(
                            out=logits_ps[t][:, b2, c, :],
                            lhsT=xt_sb[:, b, c, :],
                            rhs=wvt_sb[:, t, b2 * 64 : (b2 + 1) * 64],
                            start=False,
                            stop=False,
                            skip_group_check=True,
                        )

            for r in (1, 2):
                front(r, 1)
                front(r, 0)
                capsule(r, 1)
                capsule(r, 0)
    return nc


_NC = None


def get_nc():
    global _NC
    if _NC is None:
        _NC = build_nc()
    return _NC


def to_bf16(a):
    import ml_dtypes

    return a.astype(ml_dtypes.bfloat16)


def make_in_maps(x, weight):
    x = np.ascontiguousarray(x, dtype=np.float32)
    w = np.ascontiguousarray(weight, dtype=np.float32)  # [64, 32(l), 32(i)]
    G = np.einsum("oli,olj->oij", w, w)  # [64, 32(i), 32(i')]
    gw = np.stack([G, w], axis=1)  # [64, 2, 32, 32]
    gw = np.tile(gw, (2, 1, 1, 1))  # pair-replicated [128, 2, 32, 32]
    ident = np.eye(128, dtype=np.float32)
    in_maps = []
    for core in range(NCORES):
        xs = x[core * B : (core + 1) * B]  # [B, 1152, 32]
        xc = xs.reshape(B, C, 128, I_LEN)
        x_nat = xc.transpose(2, 0, 3, 1)  # [128, B, 32(i), C]
        x_nat = np.concatenate([x_nat, np.zeros_like(x_nat[..., :1])], axis=-1)
        xt = np.ascontiguousarray(xc.transpose(3, 0, 1, 2))  # [32, B, C, 128]
        # iteration-0: probs uniform -> s0 = colsum(x)/64; wv0 = f0 * G s0
        s0 = xs.sum(axis=1) / O_CAPS  # [B, 32]
        q0 = np.einsum("oij,bj->boi", G, s0)  # [B, 64, 32]
        ns0 = np.einsum("boi,bi->bo", q0, s0)  # [B, 64]
        f0 = np.sqrt(ns0) / (1.0 + ns0)
        wv0 = f0[:, :, None] * q0  # [B, 64, 32]
        # wvt0[i, t, b2*64+o] = wv0[2t+b2, o, i]
        wvt0 = np.empty((I_LEN, PAIRS, 2, O_CAPS), dtype=np.float32)
        for t in range(PAIRS):
            for b2 in range(2):
                wvt0[:, t, b2, :] = wv0[2 * t + b2].T
        wvt0 = wvt0.reshape(I_LEN, PAIRS, 128)
        in_maps.append(
            {
                "x_nat": to_bf16(x_nat),
                "xt": to_bf16(xt),
                "gw": to_bf16(gw),
                "ident": ident,
                "wvt0": to_bf16(wvt0),
            }
        )
    return in_maps


def assemble(results):
    outs = []
    for core in range(len(results)):
        o = np.asarray(results[core]["out"], dtype=np.float32)  # [128, PAIRS, 32]
        # v[b2*64+o, t, l] -> [b=2t+b2, o, l]
        o = o.reshape(2, O_CAPS, PAIRS, L_LEN).transpose(2, 0, 1, 3)  # [t, b2, o, l]
        outs.append(o.reshape(B, O_CAPS, L_LEN))
    return np.concatenate(outs, axis=0)


def _pin_act_table_set(nc):
    """Make Exp and Ln resolve to the one table set containing both
    (natural_log_exp_and_others), so the whole kernel runs on a single
    ACT table load."""
    from concourse.hw_specs import get_activation_tables

    tabs = get_activation_tables(nc.m.arch)
    for name, funcs in tabs.items():
        if name != "natural_log_exp_and_others":
            funcs.discard(Exp)
            funcs.discard(Ln)
            funcs.discard(mybir.ActivationFunctionType.Square)
            funcs.discard(mybir.ActivationFunctionType.Copy)
            funcs.discard(mybir.ActivationFunctionType.Identity)


def run(x, weight, trace=False):
    nc = get_nc()
    if not nc.is_finalized():
        _pin_act_table_set(nc)
        nc.finalize()
    res = run_bass_kernel_spmd(nc, make_in_maps(x, weight), list(range(NCORES)), trace=trace)
    return assemble(res.results), res


def kernel(x, weight):
    out, _ = run(x, weight)
    return out
